# revision 34
# baseline (speedup 1.0000x reference)
"""Trainium2 Bass kernel for nn_HNC_strategy (hypernetwork-conditioned MLP).

Math (per sample b):
  A[b,:]   = tanh-MLP hypernet of [t-0.5, freqs[b]]          -> [8]
  params   = A @ head_w.T + head_b                           -> [P] (never materialized)
  x[b,:]   = [cos(y[b,:64]), sin(y[b,:64])]                  -> [128]
  hid      = tanh(W1[b] @ x[b] + b1[b])                      -> [65]
  out      = W2[b] @ hid + b2[b]                             -> [64]

Key identity: W1[b] = sum_k A[b,k] * w1[:,:,k] (+ head_b part), so
  W1[b] @ x[b] = sum_k A[b,k] * (x[b] @ w1k^T)  = per-sample k-contraction
over 8 shared matmuls G_k = x @ w1k^T. Same for layer 2. The hypernet runs
feature-on-partition; G/G2 run batch-on-partition with the k-contraction on
the vector engine via scalar_tensor_tensor (per-partition scalar MAC).

All matmuls run in bf16 (fp32 is 4 cyc/row on the PE; bf16 is 1) with fp32
PSUM accumulation; the per-sample contraction stays fp32.

Sharding: pure data parallel over 8 NeuronCores (2048 rows each).
"""

import os
import sys

sys.path.insert(0, "/opt/trn_rl_repo")

import numpy as np
import ml_dtypes

import concourse.bacc as bacc
import concourse.mybir as mybir
import concourse.tile as tile
from concourse.alu_op_type import AluOpType
from concourse.bass_utils import run_bass_kernel_spmd

DIM = 64
MLPS = DIM + 1          # 65
B = 16384
H = DIM + 2             # 66
P = MLPS * 2 * DIM + MLPS + DIM * MLPS + DIM
O1 = MLPS * 2 * DIM     # 8320  end of W1 block
O2 = O1 + MLPS          # 8385  end of b1 block
O3 = O2 + DIM * MLPS    # 12545 end of W2 block
N_CORES = 8
BS = B // N_CORES       # 2048 rows per core
CH = 512                # free-dim chunk for feature-on-partition phases
NCH = BS // CH          # 4
NT = BS // 128          # 16 batch tiles per core

F32 = mybir.dt.float32
BF16 = mybir.dt.bfloat16
TANH = mybir.ActivationFunctionType.Tanh
SIN = mybir.ActivationFunctionType.Sin
BF16NP = ml_dtypes.bfloat16

_CACHE: dict = {}


def build_bass(has_hbw1: bool, has_hbw2: bool, has_hb3: bool):
    nc = bacc.Bacc("TRN2", target_bir_lowering=False, debug=False,
                   num_devices=N_CORES)

    di = lambda name, shape, dt=BF16: nc.dram_tensor(name, shape, dt,
                                                     kind="ExternalInput")
    d_zT = di("zT", [MLPS, BS])          # [t-0.5 ; freqs^T] per-core shard
    d_xT = di("xT", [2 * DIM, BS])       # [cos ; sin] host-computed, bf16
    d_w0T = di("w0T", [MLPS, H])
    d_hb0 = di("hb0", [H, 1], F32)
    d_w1T = di("w1T", [H, H])
    d_hb1 = di("hb1", [H, 1], F32)
    d_w2T = di("w2T", [H, H])
    d_hb2 = di("hb2", [H, 1], F32)
    d_w3T = di("w3T", [H, 8])
    d_hb3 = di("hb3", [8, 1], F32)
    d_hb1w = di("hb1w", [9, MLPS])       # [head_w[O1:O2].T ; head_b[O1:O2]]
    d_w3re = di("w3re", [9, DIM])        # [head_w[O3:].T ; head_b[O3:]]
    d_w1r = di("w1r", [2 * DIM, 8 * DIM])    # [i, k*64+h] = head_w[h*128+i, k], h<64
    d_w1x = di("w1x", [2 * DIM, 8])          # [i, k] = head_w[64*128+i, k]
    d_w2r = di("w2r", [MLPS, 8 * DIM])       # [h, k*64+o] = head_w[O2+o*65+h, k]
    d_ident = di("ident", [128, 128], F32)
    d_ones = di("ones", [1, BS])
    d_hbw1 = di("hbw1T", [2 * DIM, MLPS]) if has_hbw1 else None
    d_hbw2 = di("hbw2T", [MLPS, DIM]) if has_hbw2 else None
    d_hb3r = di("hb3r", [128, 8], F32) if has_hb3 else None
    d_out = nc.dram_tensor("out", [BS, DIM], F32, kind="ExternalOutput")

    mult, add = AluOpType.mult, AluOpType.add
    PSUM = "PSUM"

    with tile.TileContext(nc) as tc:
        with tc.tile_pool(name="const", bufs=1) as cp:
            # resident weights / layout constants
            w0T = cp.tile([MLPS, H], BF16)
            hb0 = cp.tile([H, 1], F32)
            w1T = cp.tile([H, H], BF16)
            hb1 = cp.tile([H, 1], F32)
            w2T = cp.tile([H, H], BF16)
            hb2 = cp.tile([H, 1], F32)
            w3T = cp.tile([H, 8], BF16)
            hb3 = cp.tile([8, 1], F32)
            hb1w = cp.tile([9, MLPS], BF16)
            w3re = cp.tile([9, DIM], BF16)
            w1r = cp.tile([2 * DIM, 8 * DIM], BF16)
            w1x = cp.tile([2 * DIM, 8], BF16)
            w2r = cp.tile([MLPS, 8 * DIM], BF16)
            ident = cp.tile([128, 128], F32)
            # spread weight loads across engine queues so the input DMAs
            # (yt/zt on sync) are not stuck behind them
            dma_engs = [nc.gpsimd]
            for i, (t_, d_) in enumerate([
                    (w0T, d_w0T), (hb0, d_hb0), (w1T, d_w1T),
                    (hb1, d_hb1), (w2T, d_w2T), (hb2, d_hb2),
                    (w3T, d_w3T), (hb3, d_hb3), (hb1w, d_hb1w),
                    (w3re, d_w3re), (w1r, d_w1r), (w1x, d_w1x),
                    (w2r, d_w2r), (ident, d_ident)]):
                dma_engs[0].dma_start(t_[:], d_[:])
            hbw1 = hbw2 = hb3r = None
            if has_hbw1:
                hbw1 = cp.tile([2 * DIM, MLPS], BF16)
                nc.sync.dma_start(hbw1[:], d_hbw1[:])
            if has_hbw2:
                hbw2 = cp.tile([MLPS, DIM], BF16)
                nc.sync.dma_start(hbw2[:], d_hbw2[:])
            if has_hb3:
                hb3r = cp.tile([128, 8], F32)
                nc.sync.dma_start(hb3r[:], d_hb3r[:])

            # persistent activations, chunk-granular so the main loop can
            # start on chunk c as soon as phase A finishes chunk c
            xTc = [cp.tile([2 * DIM, CH], BF16, name=f"xTc{c}", tag=f"xT{c}")
                   for c in range(NCH)]
            ATec = [cp.tile([9, CH], BF16, name=f"ATec{c}", tag=f"AT{c}")
                    for c in range(NCH)]
            Abc = [cp.tile([128, 32], F32, name=f"Abc{c}", tag=f"Ab{c}")
                   for c in range(NCH)]
            for c in range(NCH):
                nc.gpsimd.dma_start(ATec[c][8:9, :],
                                    d_ones[:, c * CH:(c + 1) * CH])

            # single scope: hypernet chunks (phase A) interleave with the
            # per-tile main loop so neither serializes behind the other
            with (
                tc.tile_pool(name="ld", bufs=4) as ld,
                tc.tile_pool(name="psA", bufs=1, space=PSUM) as psA,
                tc.tile_pool(name="gp", bufs=4, space=PSUM) as gp,
                tc.tile_pool(name="cb", bufs=2, space=PSUM) as cb,
                tc.tile_pool(name="tp", bufs=1, space=PSUM) as tp,
                tc.tile_pool(name="sb", bufs=4) as sb,
                tc.tile_pool(name="ob", bufs=2) as obp,
            ):
                for c in range(NCH):
                    sl = slice(c * CH, (c + 1) * CH)
                    nc.sync.dma_start(xTc[c][:], d_xT[:, sl])

                def phaseA(c):
                    sl = slice(c * CH, (c + 1) * CH)
                    zt = ld.tile([MLPS, CH], BF16, tag="zt")
                    nc.sync.dma_start(zt[:], d_zT[:, sl])
                    p0 = psA.tile([H, CH], F32, tag="hp")
                    nc.tensor.matmul(p0[:], w0T[:], zt[:])
                    h0 = ld.tile([H, CH], BF16, tag="h")
                    nc.scalar.activation(h0[:], p0[:], TANH, bias=hb0[:])
                    p1 = psA.tile([H, CH], F32, tag="hp")
                    nc.tensor.matmul(p1[:], w1T[:], h0[:])
                    h1 = ld.tile([H, CH], BF16, tag="h")
                    nc.scalar.activation(h1[:], p1[:], TANH, bias=hb1[:])
                    p2 = psA.tile([H, CH], F32, tag="hp")
                    nc.tensor.matmul(p2[:], w2T[:], h1[:])
                    h2 = ld.tile([H, CH], BF16, tag="h")
                    nc.scalar.activation(h2[:], p2[:], TANH, bias=hb2[:])
                    pAT = psA.tile([8, CH], F32, tag="hp")
                    nc.tensor.matmul(pAT[:], w3T[:], h2[:])
                    nc.scalar.activation(ATec[c][0:8, :], pAT[:], TANH,
                                         bias=hb3[:])
                    for j4 in range(4):
                        pA = psA.tile([128, 8], F32, tag="hp")
                        nc.tensor.matmul(pA[:], h2[:, j4 * 128:(j4 + 1) * 128],
                                         w3T[:])
                        if has_hb3:
                            nc.vector.tensor_add(pA[:], pA[:], hb3r[:])
                        nc.scalar.activation(Abc[c][:, j4 * 8:(j4 + 1) * 8],
                                             pA[:], TANH)

                # per-tile: bilinear head, k-contraction, layer 2.
                # G = x @ w_r -> PSUM bank; DVE: M = G * A_bcast (1x op,
                # bf16 out) then a binary tree of bf16 adds (2x) over k.
                st = {}   # per-tile carried state

                def refs(j):
                    c, r = j // 4, j % 4
                    rsl = slice(r * 128, (r + 1) * 128)
                    return (xTc[c][:, rsl], ATec[c][:, rsl],
                            Abc[c][:, r * 8:(r + 1) * 8])

                def stage1(j):
                    xTj, ATj, Abj = refs(j)
                    g1 = gp.tile([128, 8 * DIM], F32, tag="g")
                    nc.tensor.matmul(g1[:], xTj, w1r[:])
                    # c1p/g1x/c2p for three tiles share one PSUM bank
                    if j % 3 == 0:
                        st["ccb"] = cb.tile([128, 432], F32, name="ccb",
                                            tag="cb")
                    ccb = st["ccb"]
                    base = (j % 3) * 144
                    cc = ccb[:, base:base + 144]
                    nc.tensor.matmul(cc[:, 72:80], xTj, w1x[:])
                    nc.tensor.matmul(cc[:, 0:MLPS], ATj, hb1w[:],
                                     start=True, stop=not has_hbw1)
                    if has_hbw1:
                        nc.tensor.matmul(cc[:, 0:MLPS], xTj, hbw1[:],
                                         start=False, stop=True)
                    # c2p only needs A^T; emit here so stage2's PE chain is
                    # just transpose+g2
                    nc.tensor.matmul(cc[:, 80:144], ATj, w3re[:],
                                     start=True, stop=not has_hbw2)

                    m1 = sb.tile([128, 8 * DIM], BF16, tag="m")
                    nc.vector.tensor_tensor(
                        m1[:].rearrange("p (k h) -> p k h", k=8),
                        g1[:].rearrange("p (k h) -> p k h", k=8),
                        Abj.unsqueeze(-1).broadcast_to([128, 8, DIM]),
                        mult)
                    t1 = sb.tile([128, 4 * DIM], BF16, tag="t1")
                    nc.vector.tensor_add(t1[:], m1[:, 0:256], m1[:, 256:512])
                    t2 = sb.tile([128, 2 * DIM], BF16, tag="t2")
                    nc.vector.tensor_add(t2[:], t1[:, 0:128], t1[:, 128:256])
                    t3 = sb.tile([128, DIM], BF16, tag="t3")
                    nc.vector.tensor_add(t3[:], t2[:, 0:DIM], t2[:, DIM:128])

                    hp = sb.tile([128, MLPS], F32, tag="hp")
                    nc.vector.tensor_add(hp[:, 0:DIM], t3[:], cc[:, 0:DIM])
                    # 65th hidden unit: hp[:,64] = c1p_x + sum_k A_k*G1x_k
                    junk = sb.tile([128, 8], F32, tag="junk")
                    xacc = sb.tile([128, 1], F32, tag="xacc")
                    nc.vector.scalar_tensor_tensor(
                        junk[:], cc[:, 72:80], 1.0, Abj, mult, mult,
                        accum_out=xacc[:])
                    nc.vector.scalar_tensor_tensor(
                        hp[:, DIM:MLPS], cc[:, DIM:MLPS], 1.0, xacc[:],
                        mult, add)
                    st[j] = (hp, cc, ATj, Abj)

                def stage2(j):
                    hp, cc, ATj, Abj = st.pop(j)
                    c, r = j // 4, j % 4
                    # transpose fp32 pre-activation, tanh straight out of
                    # PSUM (ScalarE is PSUM-adjacent) with bf16 cast
                    tpp = tp.tile([MLPS, 128], F32, tag="tp")
                    nc.tensor.transpose(tpp[:], hp[:], ident[:])
                    hidT = sb.tile([MLPS, 128], BF16, tag="hidT")
                    nc.scalar.activation(hidT[:], tpp[:], TANH)

                    g2 = gp.tile([128, 8 * DIM], F32, tag="g")
                    nc.tensor.matmul(g2[:], hidT[:], w2r[:])
                    if has_hbw2:
                        nc.tensor.matmul(cc[:, 80:144], hidT[:], hbw2[:],
                                         start=False, stop=True)

                    m2 = sb.tile([128, 8 * DIM], BF16, tag="m")
                    nc.vector.tensor_tensor(
                        m2[:].rearrange("p (k o) -> p k o", k=8),
                        g2[:].rearrange("p (k o) -> p k o", k=8),
                        Abj.unsqueeze(-1).broadcast_to([128, 8, DIM]),
                        mult)
                    u1 = sb.tile([128, 4 * DIM], BF16, tag="t1")
                    nc.vector.tensor_add(u1[:], m2[:, 0:256], m2[:, 256:512])
                    u2 = sb.tile([128, 2 * DIM], BF16, tag="t2")
                    nc.vector.tensor_add(u2[:], u1[:, 0:128], u1[:, 128:256])
                    if r == 0:
                        st["og"] = obp.tile([128, 4 * DIM], F32, name="og",
                                            tag="og")
                    o_g = st["og"]
                    osl = slice(r * DIM, (r + 1) * DIM)
                    nc.vector.scalar_tensor_tensor(
                        o_g[:, osl], u2[:, 0:DIM], 1.0, u2[:, DIM:128],
                        mult, add)
                    nc.vector.tensor_add(o_g[:, osl], o_g[:, osl],
                                         cc[:, 80:144])
                    if r == 3:
                        # one DMA per 4-tile group; row (t*128+b) <- (b, t*64+o)
                        dst = d_out[c * 512:(c + 1) * 512, :].rearrange(
                            "(t b) o -> b t o", t=4)
                        nc.gpsimd.dma_start(
                            dst, o_g[:].rearrange("p (t o) -> p t o", t=4))

                # depth-2 software pipeline, interleaved with hypernet
                # chunks: tiles of chunk c-1 are emitted right after
                # phaseA(c) so main work fills phase-A stalls and vice versa
                for c in range(NCH):
                    phaseA(c)
                    if c >= 1:
                        for r in range(4):
                            j = 4 * (c - 1) + r
                            stage1(j)
                            if j >= 2:
                                stage2(j - 2)
                for j in range(4 * (NCH - 1), NT):
                    stage1(j)
                    stage2(j - 2)
                stage2(NT - 2)
                stage2(NT - 1)

    nc.compile()
    return nc


def _prep(inputs):
    f = lambda name: np.ascontiguousarray(
        np.asarray(inputs[name], dtype=np.float32))
    t = float(np.asarray(inputs["t"]))
    y, freqs = f("y"), f("freqs")
    hw0, hb0 = f("hw0"), f("hb0")
    hw1, hb1 = f("hw1"), f("hb1")
    hw2, hb2 = f("hw2"), f("hb2")
    hw3, hb3 = f("hw3"), f("hb3")
    head_w, head_b = f("head_w"), f("head_b")

    zT = np.empty((MLPS, B), np.float32)
    zT[0, :] = t - 0.5
    zT[1:, :] = freqs.T
    y64 = y[:, :DIM].astype(np.float64)
    xT = np.empty((2 * DIM, B), BF16NP)
    xT[:DIM, :] = np.cos(y64).T.astype(BF16NP)
    xT[DIM:, :] = np.sin(y64).T.astype(BF16NP)

    C = np.ascontiguousarray
    b16 = lambda a: np.ascontiguousarray(np.asarray(a, dtype=BF16NP))
    w1t = head_w[:O1].reshape(MLPS, 2 * DIM, 8)
    w1r = b16(w1t[:DIM].transpose(1, 2, 0).reshape(2 * DIM, 8 * DIM))
    w1x = b16(w1t[DIM])                          # [2*DIM, 8]
    w2r = b16(head_w[O2:O3].reshape(DIM, MLPS, 8)
              .transpose(1, 2, 0).reshape(MLPS, 8 * DIM))
    hb1w = b16(np.concatenate([head_w[O1:O2].T, head_b[None, O1:O2]], axis=0))
    w3re = b16(np.concatenate([head_w[O3:].T, head_b[None, O3:]], axis=0))
    hbw1 = C(head_b[:O1].reshape(MLPS, 2 * DIM).T)
    hbw2 = C(head_b[O2:O3].reshape(DIM, MLPS).T)

    has_hbw1 = bool(np.any(hbw1))
    has_hbw2 = bool(np.any(hbw2))
    has_hb3 = bool(np.any(hb3))

    shared = {
        "w0T": b16(hw0.T), "hb0": C(hb0[:, None]),
        "w1T": b16(hw1.T), "hb1": C(hb1[:, None]),
        "w2T": b16(hw2.T), "hb2": C(hb2[:, None]),
        "w3T": b16(hw3.T), "hb3": C(hb3[:, None]),
        "hb1w": hb1w, "w3re": w3re, "w1r": w1r, "w1x": w1x, "w2r": w2r,
        "ident": np.eye(128, dtype=np.float32),
        "ones": np.ones((1, BS), BF16NP),
    }
    if has_hbw1:
        shared["hbw1T"] = b16(hbw1)
    if has_hbw2:
        shared["hbw2T"] = b16(hbw2)
    if has_hb3:
        shared["hb3r"] = C(np.tile(hb3[None, :], (128, 1)))

    in_maps = []
    for c in range(N_CORES):
        sl = slice(c * BS, (c + 1) * BS)
        in_maps.append({
            **shared,
            "zT": b16(zT[:, sl]),
            "xT": np.ascontiguousarray(xT[:, sl]),
        })
    return in_maps, (has_hbw1, has_hbw2, has_hb3)


def _run(inputs, trace=False):
    in_maps, flags = _prep(inputs)
    if flags not in _CACHE:
        _CACHE[flags] = build_bass(*flags)
    nc = _CACHE[flags]
    res = run_bass_kernel_spmd(nc, in_maps, core_ids=list(range(N_CORES)),
                               trace=trace)
    out = np.concatenate([r["out"] for r in res.results], axis=0)
    return out, res


def kernel(**inputs) -> np.ndarray:
    out, _ = _run(inputs)
    return out


if __name__ == "__main__":
    rng = np.random.default_rng(0)
    demo = {
        "t": np.float32(0.3),
        "y": rng.standard_normal((B, 2 * DIM), dtype=np.float32),
        "freqs": rng.random((B, DIM), dtype=np.float32),
        "hw0": rng.standard_normal((H, 1 + DIM), dtype=np.float32) * 0.05,
        "hb0": np.zeros(H, np.float32),
        "hw1": rng.standard_normal((H, H), dtype=np.float32) * 0.05,
        "hb1": np.zeros(H, np.float32),
        "hw2": rng.standard_normal((H, H), dtype=np.float32) * 0.05,
        "hb2": np.zeros(H, np.float32),
        "hw3": rng.standard_normal((8, H), dtype=np.float32) * 0.05,
        "hb3": np.zeros(8, np.float32),
        "head_w": rng.standard_normal((P, 8), dtype=np.float32) * 0.05,
        "head_b": np.zeros(P, np.float32),
    }
    out = kernel(**demo)
    print("out", out.shape, out.dtype, float(np.abs(out).max()))
